# revision 49
# baseline (speedup 1.0000x reference)
"""Trainium2 Bass kernel for nn_Att6 (attention-pooling block).

Computes, for each batch b:
    ht  = tanh(t[b] @ wt)                     (T, H)
    c   = tanh(a[b] @ wa) * tanh(b[b] @ wb) * wh[:, 0]        (H,)
    s   = ht @ c                              (T,)   scores
    att = softmax(s) * mask; att /= sum(att)  (T,)
    out = att @ t[b]                          (D,)

Sharding: data-parallel over batch B=32 across 8 NeuronCores (4 batches
per core), weights replicated.

Precision/layout strategy:
  - t is converted to bf16 on the host; the kernel reads it natively
    (tau on partitions, for pooling) and transposed (d on partitions,
    for the big matmul) via the DMA crossbar transpose -- no PE
    transposes for t.
  - mm1 (t @ wt) runs in fp8e4m3 with DoubleRow perf mode (2 k-tiles
    per instruction), 2x the bf16/f32r rate on HW.  wt is pre-scaled by
    WT_SCALE on the host so its values sit in fp8's normal range; the
    tanh activation divides the scale back out.
  - scores (mm2) and pooling stay in f32r/bf16 for accuracy.
"""

import sys

sys.path.insert(0, "/opt/trn_rl_repo")

import numpy as np

import bass_rust
import concourse.bass as bass
import concourse.tile as tile
from concourse import mybir
from concourse.masks import make_identity

F32 = mybir.dt.float32
F32R = mybir.dt.float32r
BF16 = mybir.dt.bfloat16
F8 = mybir.dt.float8e4
AF = mybir.ActivationFunctionType
AX = mybir.AxisListType
DR = mybir.MatmulPerfMode.DoubleRow

WT_SCALE = 32.0
DEBUG_DUMP = False  # dump batch-1 tT16/tT8 to DRAM for layout debugging
MM1_MODE = "fp8dr"  # "fp8dr" (DoubleRow fp8) or "bf16" (no conversion)
TXP_MODE = "pe"     # "xbar" (DMA crossbar) or "pe" (PE transposes of t_nat)

N_CORES = 8
B, T, D, H = 32, 2048, 1024, 1024
BL = B // N_CORES            # batches per core
TCH = 512                    # tau-chunk (columns of one moving matmul)
NTCH = T // TCH              # 4 chunks per batch
NTT = TCH // 128             # 4 tau-tiles per chunk
KD = D // 128                # 8 contraction chunks over D
KH = H // 128                # 8 chunks over H


def split_sync_waits(nc, max_waits=1):
    """This container's walrus accepts only one sem-wait per instruction.
    Move extra waits onto same-engine NOPs inserted immediately before."""
    n_new = 0
    for f in nc.m.functions:
        for bb in f.blocks:
            new = []
            for inst in bb.instructions:
                si = inst.sync_info
                waits = list(si.on_wait) if (si and si.on_wait) else []
                if len(waits) > max_waits:
                    extra, keep = waits[:-max_waits], waits[-max_waits:]
                    for w in extra:
                        nop = bass_rust.InstNoOp(
                            name=f"{inst.name}-sw{n_new}", ins=[], outs=[])
                        nop.engine = inst.engine
                        nop.sync_info = mybir.SyncInfo(on_wait=[w], on_update=[])
                        new.append(nop)
                        n_new += 1
                    si.on_wait = keep
                new.append(inst)
            bb.instructions[:] = new
    return n_new


def build_nc(split_waits=True, reps=1):
    nc = bass.Bass()
    t_in = nc.declare_dram_parameter("t", [BL, T, D], BF16, isOutput=False)
    a_in = nc.declare_dram_parameter("a", [BL, D], F32, isOutput=False)
    b_in = nc.declare_dram_parameter("b", [BL, D], F32, isOutput=False)
    m_in = nc.declare_dram_parameter("mask", [BL, T], F32, isOutput=False)
    wt_in = nc.declare_dram_parameter("wt", [D, H], F32, isOutput=False)
    wa_in = nc.declare_dram_parameter("wa", [D, H], F32, isOutput=False)
    wb_in = nc.declare_dram_parameter("wb", [D, H], F32, isOutput=False)
    wh_in = nc.declare_dram_parameter("wh", [H], F32, isOutput=False)
    out_d = nc.declare_dram_parameter("out", [BL, D], F32, isOutput=True)
    dbg = None
    if DEBUG_DUMP:
        dbg = (nc.declare_dram_parameter("dbg16", [128, KD, T], BF16,
                                         isOutput=True),
               nc.declare_dram_parameter("dbg8", [128, KD, TCH],
                                         mybir.dt.uint8, isOutput=True))

    with tile.TileContext(nc) as tc:
        _body(nc, tc, t_in, a_in, b_in, m_in, wt_in, wa_in, wb_in, wh_in,
              out_d, reps, dbg)
    if split_waits:
        split_sync_waits(nc)
    return nc


def _body(nc, tc, t_in, a_in, b_in, m_in, wt_in, wa_in, wb_in, wh_in, out_d,
          reps, dbg=None):
    with (
        tc.tile_pool(name="const", bufs=1) as const,
        tc.tile_pool(name="wts", bufs=1) as wts,
        tc.tile_pool(name="wtstage", bufs=2) as wtstage,
        tc.tile_pool(name="wab", bufs=4) as wab,
        tc.tile_pool(name="small", bufs=1) as small,
        tc.tile_pool(name="tch", bufs=5 if TXP_MODE == "pe" else 3) as tch,
        tc.tile_pool(name="tT16", bufs=2) as tT16p,
        tc.tile_pool(name="tT8", bufs=2) as tT8p,
        tc.tile_pool(name="hT", bufs=6 if TXP_MODE == "pe" else 4) as hTp,
        tc.tile_pool(name="rows", bufs=2) as rows,
        tc.tile_pool(name="rowsm", bufs=2) as rowsm,
        tc.tile_pool(name="ps_tr", bufs=2, space="PSUM") as ps_tr,
        tc.tile_pool(name="ps_mm", bufs=2, space="PSUM") as ps_mm,
        tc.tile_pool(name="ps_row", bufs=2, space="PSUM") as ps_row,
    ):
        ident = const.tile([128, 128], F32)
        make_identity(nc, ident)
        identb = None
        if TXP_MODE == "pe":
            identb = const.tile([128, 128], BF16, tag="identb")
            nc.vector.tensor_copy(identb, ident)

        # tiny vector loads + transposes first so PE work exists early
        vT = {}
        for name, vec_in in (("a", a_in), ("b", b_in)):
            v_nat = small.tile([BL, D], F32, tag=f"v{name}")
            nc.sync.dma_start(out=v_nat, in_=vec_in[:, :])
            vT_sb = small.tile([128, KD, BL], F32R, tag=f"vT{name}")
            vT[name] = vT_sb
            for k in range(KD):
                ps = ps_tr.tile([128, BL], F32, tag="tr")
                nc.tensor.transpose(
                    ps, v_nat[:, k * 128:(k + 1) * 128], ident[:BL, :BL])
                nc.vector.tensor_copy(vT_sb[:, k, :], ps)

        def emit_chunk_dma(b, j, halves=1):
            t_nat = tch.tile([128, NTT, D], BF16, tag="tch", name="t_nat")
            hs = NTT // halves
            for h0 in range(0, NTT, hs):
                nc.sync.dma_start(
                    out=t_nat[:, h0:h0 + hs, :],
                    in_=t_in[b, (j * NTT + h0) * 128:(j * NTT + h0 + hs) * 128, :]
                    .rearrange("(tt p) d -> p tt d", p=128))
            return t_nat

        def emit_batch_txp(b):
            # transposed load of the whole batch straight from DRAM via the
            # DMA crossbar: tT16[p, k, tau] = t[b, tau, k*128+p].  One big
            # DMA per k-slice (128 xbar tiles each) keeps the per-DMA HWDGE
            # overhead (~630ns) amortized.
            tT16 = tT16p.tile([128, KD, T], BF16, tag="tT16", name="tT16")
            for k in range(KD):
                nc.sync.dma_start(
                    out=tT16[:, k, :],
                    in_=t_in[b, :, k * 128:(k + 1) * 128],
                    transpose=True)
            return tT16

        def emit_chunk_fp8(tT16, j):
            if MM1_MODE == "bf16":
                return tT16  # mm1 reads bf16 slices of the batch tile
            tT8 = tT8p.tile([128, KD, TCH], F8, tag="tT8", name="tT8")
            sl = slice(j * TCH, (j + 1) * TCH)
            for k in range(KD):
                eng = nc.vector if k % 2 == 0 else nc.gpsimd
                eng.tensor_copy(tT8[:, k, :], tT16[:, k, sl])
            return tT8

        def emit_chunk_txp_pe(t_nat):
            # PE transposes (bf16, 1.0 cycles/row) of the native chunk, fp8
            # conversion fused into the PSUM->SBUF copy
            tT8 = tT8p.tile([128, KD, TCH], F8, tag="tT8", name="tT8")
            for k in range(KD):
                ps = ps_tr.tile([128, TCH], BF16, tag="tr", name="ps_trb")
                for tt in range(NTT):
                    nc.tensor.transpose(
                        ps[:, tt * 128:(tt + 1) * 128],
                        t_nat[:, tt, k * 128:(k + 1) * 128], identb)
                # Pool (gpsimd) cannot read PSUM; split DVE/Act instead
                if k % 2 == 0:
                    nc.vector.tensor_copy(tT8[:, k, :], ps)
                else:
                    nc.scalar.copy(tT8[:, k, :], ps)
            return tT8

        txp_tiles = {}
        nat_pre = {(0, 0): emit_chunk_dma(0, 0)}
        if TXP_MODE == "xbar":
            txp_tiles[0] = emit_batch_txp(0)
            tT8_pre = {(0, 0): emit_chunk_fp8(txp_tiles[0], 0)}
        else:
            tT8_pre = {(0, 0): emit_chunk_txp_pe(nat_pre[(0, 0)])}

        # weight DMA order: per h-chunk, wa/wb pair (phase 0 consumes them
        # early) interleaved with the matching wt h-slice
        w_tiles = {}
        predma = {}
        wt_sb = wts.tile([128, KD, H], F8 if MM1_MODE == "fp8dr" else BF16)
        for hh in range(KH):
            # weight DMAs ride the Activation HWDGE queue so the startup
            # weight flood never shares DMA queues with the xbar transposes
            # (sharing corrupts the xbar output on HW)
            for name, w_in in (("a", wa_in), ("b", wb_in)):
                w_sb = wab.tile(
                    [128, KD, 128], F32R, tag="wsb", name=f"w{name}{hh}")
                nc.scalar.dma_start(
                    out=w_sb,
                    in_=w_in[:, hh * 128:(hh + 1) * 128]
                    .bitcast(F32R).rearrange("(k p) h -> p k h", p=128))
                w_tiles[(name, hh)] = w_sb
            # one-time fp8 weight quantization via a small rotating f32
            # stage (amortized across reps)
            wstage = wtstage.tile([128, KD, 128], F32R, tag="ws",
                                  name=f"ws{hh}")
            nc.scalar.dma_start(
                out=wstage,
                in_=wt_in[:, hh * 128:(hh + 1) * 128]
                .bitcast(F32R).rearrange("(k p) h -> p k h", p=128))
            nc.vector.tensor_copy(
                wt_sb[:, :, hh * 128:(hh + 1) * 128], wstage)
            if hh == 2:
                # slip batch0-chunk1's t DMA into the weight stream so its
                # data is resident when chunk0's compute finishes
                nat_pre[(0, 1)] = emit_chunk_dma(0, 1)

        whT_sb = const.tile([128, KH], F32)
        nc.scalar.dma_start(out=whT_sb,
                            in_=wh_in.rearrange("(k p) -> p k", p=128))

        # ---- phase 0 (h-chunked, interleaved into the first chunk's hh
        # loop): c = tanh(a@wa) * tanh(b@wb) * wh ----
        cT_sb = small.tile([128, KH, BL], F32R)

        def emit_phase0_hh(hh):
            hv = {}
            for name in ("a", "b"):
                w_sb = w_tiles.pop((name, hh))
                ps = ps_row.tile([128, BL], F32, tag="s")
                for k in range(KD):
                    nc.tensor.matmul(
                        ps, w_sb[:, k, :], vT[name][:, k, :],
                        start=(k == 0), stop=(k == KD - 1))
                hv[name] = wab.tile(
                    [128, BL], F32, tag=f"h{name}", name=f"h{name}")
                nc.scalar.activation(hv[name], ps, AF.Tanh)
            prod = wab.tile([128, BL], F32, tag="prod")
            nc.vector.tensor_mul(prod, hv["a"], hv["b"])
            nc.vector.tensor_mul(
                cT_sb[:, hh, :], prod,
                whT_sb[:, hh:hh + 1].to_broadcast([128, BL]))

        # ---- main loop: per chunk, scores -> exp -> mask -> partial
        # pooling accumulate; t chunks release immediately.  No score-max
        # subtraction: |s| <= ||wh||_1 ~ 36 << 88, so exp cannot overflow.
        seq = [(rep, b) for rep in range(reps) for b in range(BL)]
        deferred = [None]

        def flush_deferred():
            if deferred[0] is not None:
                fn = deferred[0]
                deferred[0] = None
                fn()

        def make_pool_partial(b, j, t_nat, att_b, ps_out, den_parts, finalize):
            def fn():
                # transpose the 4 e-columns, accumulate the pooling matmul
                attT = rowsm.tile([128, NTT], BF16, tag="attT", name="attT")
                ps_a = ps_tr.tile([128, NTT], F32, tag="tr", name="ps_a")
                for tt in range(NTT):
                    i = j * NTT + tt
                    nc.tensor.transpose(
                        ps_a[:, tt:tt + 1],
                        att_b[:, i * 128:(i + 1) * 128], ident[:1, :1])
                nc.vector.tensor_copy(attT, ps_a)
                for dh in range(2):
                    for tt in range(NTT):
                        nc.tensor.matmul(
                            ps_out[dh], attT[:, tt:tt + 1],
                            t_nat[:, tt, dh * TCH:(dh + 1) * TCH],
                            start=(j == 0 and tt == 0),
                            stop=(j == NTCH - 1 and tt == NTT - 1),
                            skip_group_check=True)
                if finalize:
                    den = rowsm.tile([1, 1], F32, tag="den", name="den")
                    nc.vector.reduce_sum(
                        out=den, in_=den_parts[:, :], axis=AX.X)
                    rden = rowsm.tile([1, 1], F32, tag="rden", name="rden")
                    nc.vector.reciprocal(rden, den)
                    out_b = rows.tile([1, D], F32, tag="orow", name="out_b")
                    for dh in range(2):
                        nc.vector.tensor_scalar_mul(
                            out_b[:, dh * TCH:(dh + 1) * TCH], ps_out[dh], rden)
                    nc.sync.dma_start(out=out_d[b:b + 1, :], in_=out_b)
            return fn

        masks = {}
        for idx, (rep, b) in enumerate(seq):
            if idx in masks:
                mask_b = masks.pop(idx)
            else:
                mask_b = rows.tile([1, T], F32, tag="mrow")
                nc.sync.dma_start(out=mask_b, in_=m_in[b:b + 1, :])
            att_b = rows.tile([1, T], F32, tag="arow")
            den_parts = rowsm.tile([1, NTCH], F32, tag="denp")
            ps_out = [ps_row.tile([1, TCH], F32, tag="o", name=f"o{dh}")
                      for dh in range(2)]
            for j in range(NTCH):
                key = (idx, j)
                t_nat = nat_pre.pop(key)
                tT8 = tT8_pre.pop(key)

                # --- prefetches for the NEXT chunk, ahead of this chunk's
                # PE work (Tile keeps per-engine emission order, so these
                # must be queued before the matmuls that would hide them)
                nxt = (idx, j + 1) if j + 1 < NTCH else (idx + 1, 0)
                if nxt[0] < len(seq):
                    nb = seq[nxt[0]][1]
                    if TXP_MODE == "xbar" and j == 1 and idx + 1 < len(seq):
                        # next batch's transposed load (8 big xbar DMAs)
                        txp_tiles[idx + 1] = emit_batch_txp(seq[idx + 1][1])
                    if nxt not in nat_pre:
                        nat_pre[nxt] = emit_chunk_dma(nb, nxt[1])
                    if TXP_MODE == "pe":
                        # two chunks of native lookahead: covers DMA latency
                        # so the PE transposes never stall (HAM throttling)
                        nx2 = ((nxt[0], nxt[1] + 1) if nxt[1] + 1 < NTCH
                               else (nxt[0] + 1, 0))
                        if nx2[0] < len(seq) and nx2 not in nat_pre:
                            nat_pre[nx2] = emit_chunk_dma(
                                seq[nx2[0]][1], nx2[1])
                    if TXP_MODE == "xbar" and nxt not in tT8_pre:
                        tT8_pre[nxt] = emit_chunk_fp8(
                            txp_tiles[nxt[0]], nxt[1])
                        if dbg is not None and nxt == (1, 0):
                            nc.sync.dma_start(out=dbg[0][:, :, :],
                                              in_=txp_tiles[1])
                            nc.sync.dma_start(
                                out=dbg[1][:, :, :],
                                in_=tT8_pre[nxt].bitcast(mybir.dt.uint8))
                    if j == 2 and idx + 1 < len(seq):
                        mrow = rows.tile([1, T], F32, tag="mrow")
                        nc.sync.dma_start(
                            out=mrow, in_=m_in[seq[idx + 1][1]:
                                               seq[idx + 1][1] + 1, :])
                        masks[idx + 1] = mrow
                if j == NTCH - 1:
                    txp_tiles.pop(idx, None)

                sl = slice(j * TCH, (j + 1) * TCH)
                # next chunk's transposed fp8 tile is filled one k-slice per
                # hh iteration below, so the PE transpose cost is spread in
                # small slices and each PSUM->SBUF copy has a full hh-period
                # to drain (a bursty tail stalls the PE on copy throughput)
                tT8_next = None
                if (TXP_MODE == "pe" and nxt[0] < len(seq)
                        and nxt not in tT8_pre):
                    tT8_next = tT8p.tile([128, KD, TCH], F8, tag="tT8",
                                         name="tT8n")
                    tT8_pre[nxt] = tT8_next
                    t_nat_next = nat_pre[nxt]
                    sl_next = slice(nxt[1] * TCH, (nxt[1] + 1) * TCH)
                ps_s = ps_row.tile([1, TCH], F32, tag="s")
                for hh in range(KH):
                    ps_h = ps_mm.tile([128, TCH], F32, tag="mm1")
                    if MM1_MODE == "bf16":
                        for k in range(KD):
                            nc.tensor.matmul(
                                ps_h,
                                wt_sb[:, k, hh * 128:(hh + 1) * 128],
                                tT8[:, k, sl],
                                start=(k == 0), stop=(k == KD - 1))
                    else:
                      for k2 in range(KD // 2):
                        nc.tensor.matmul(
                            ps_h,
                            wt_sb[:, 2 * k2:2 * k2 + 2,
                                  hh * 128:(hh + 1) * 128],
                            tT8[:, 2 * k2:2 * k2 + 2, :],
                            start=(k2 == 0), stop=(k2 == KD // 2 - 1),
                            perf_mode=DR)
                    hT = hTp.tile([128, TCH], F32R, tag="hT")
                    nc.scalar.activation(hT, ps_h, AF.Tanh,
                                         scale=1.0 / WT_SCALE)
                    if idx == 0 and j == 0:
                        emit_phase0_hh(hh)
                    if tT8_next is not None:
                        # independent PE filler between mm1(hh) and mm2(hh):
                        # absorbs the tanh latency mm2 would otherwise wait on
                        ps_t = ps_tr.tile([128, TCH], BF16, tag="tr",
                                          name="ps_trb")
                        for tt in range(NTT):
                            nc.tensor.transpose(
                                ps_t[:, tt * 128:(tt + 1) * 128],
                                t_nat_next[:, tt, hh * 128:(hh + 1) * 128],
                                identb)
                        if hh % 4 == 3:
                            nc.scalar.copy(tT8_next[:, hh, :], ps_t)
                        else:
                            nc.vector.tensor_copy(tT8_next[:, hh, :], ps_t)
                    nc.tensor.matmul(
                        ps_s, cT_sb[:, hh, b:b + 1], hT,
                        start=(hh == 0), stop=(hh == KH - 1),
                        skip_group_check=True)

                # mask folded into the scores as an additive bias (host
                # passes (m-1)*50, so exp(s+bias) = exp(s)*m to ~1e-21);
                # the exp's accum_out yields the denominator for free
                nc.vector.tensor_add(ps_s, ps_s, mask_b[:, sl])
                nc.scalar.activation(att_b[:, sl], ps_s, AF.Exp,
                                     accum_out=den_parts[:, j:j + 1])

                flush_deferred()
                deferred[0] = make_pool_partial(
                    b, j, t_nat, att_b, ps_out, den_parts,
                    finalize=(j == NTCH - 1))
        flush_deferred()


_NC = None


def _get_nc():
    global _NC
    if _NC is None:
        _NC = build_nc()
    return _NC


def _shard_inputs(t, a, b, mask, wt, wa, wb, wh):
    import ml_dtypes

    t16 = np.asarray(t, dtype=np.float32).astype(ml_dtypes.bfloat16)
    a = np.asarray(a, dtype=np.float32)
    b = np.asarray(b, dtype=np.float32)
    # additive mask bias: exp(s + (m-1)*50) == exp(s)*m to fp32 precision
    mask_f = (np.asarray(mask).astype(np.float32) - 1.0) * 50.0
    wt = np.ascontiguousarray(
        np.asarray(wt, dtype=np.float32) * np.float32(WT_SCALE))
    wa = np.ascontiguousarray(np.asarray(wa, dtype=np.float32))
    wb = np.ascontiguousarray(np.asarray(wb, dtype=np.float32))
    wh = np.ascontiguousarray(np.asarray(wh, dtype=np.float32).reshape(H))
    in_maps = []
    for c in range(N_CORES):
        sl = slice(BL * c, BL * (c + 1))
        in_maps.append({
            "t": np.ascontiguousarray(t16[sl]),
            "a": np.ascontiguousarray(a[sl]),
            "b": np.ascontiguousarray(b[sl]),
            "mask": np.ascontiguousarray(mask_f[sl]),
            "wt": wt, "wa": wa, "wb": wb, "wh": wh,
        })
    return in_maps


def kernel(t, a, b, mask, wt, wa, wb, wh):
    from concourse.bass_utils import run_bass_kernel_spmd

    nc = _get_nc()
    in_maps = _shard_inputs(t, a, b, mask, wt, wa, wb, wh)
    res = run_bass_kernel_spmd(nc, in_maps, core_ids=list(range(N_CORES)))
    out = np.concatenate([res.results[c]["out"] for c in range(N_CORES)], axis=0)
    return np.ascontiguousarray(out, dtype=np.float32)
